# revision 5
# baseline (speedup 1.0000x reference)
"""HRNetV2 backbone on Trainium2 (Bass/Tile), data-parallel over 8 NeuronCores.

Layout: activations channel-on-partitions, spatially padded [C, H+2, W+2] fp16.
Convs: weight-stationary matmuls (lhsT=[Cin,Cout] tap tiles), fp32 PSUM accum.
BN folded to scale/shift on host; bilinear upsampling via 3-tap phase-split DVE.
"""
import sys
sys.path.insert(0, "/opt/trn_rl_repo")
import numpy as np

EPS = 1e-5
P = 128


def _np(t):
    return np.asarray(t, dtype=np.float32)


class Col:
    """Two-pass collector: pass1 records host arrays, pass2 replays indices."""

    def __init__(self):
        self.wtiles = []
        self.bns = []
        self.uw = {}
        self.uwsegs = []
        self.uwlen = 0
        self.pass2 = False
        self.wcur = 0
        self.bcur = 0

    def conv(self, w, Cout, Cin, k):
        widx = {}
        bslots = []
        n_mt = (Cout + P - 1) // P
        n_ct = (Cin + P - 1) // P
        for mt in range(n_mt):
            if self.pass2:
                bslots.append(self.bcur)
                self.bcur += 1
            co0, co1 = mt * P, min((mt + 1) * P, Cout)
            for ky in range(k):
                for kx in range(k):
                    for ct in range(n_ct):
                        if self.pass2:
                            widx[(mt, ky, kx, ct)] = self.wcur
                            self.wcur += 1
                        else:
                            ci0, ci1 = ct * P, min((ct + 1) * P, Cin)
                            t = np.zeros((P, P), np.float16)
                            t[: ci1 - ci0, : co1 - co0] = (
                                w[co0:co1, ci0:ci1, ky, kx].T.astype(np.float16))
                            self.wtiles.append(t)
        return widx, bslots

    def bn(self, g, b, m, v, Cout):
        scale = _np(g) / np.sqrt(_np(v) + EPS)
        shift = _np(b) - _np(m) * scale
        for mt in range((Cout + P - 1) // P):
            co0, co1 = mt * P, min((mt + 1) * P, Cout)
            bn = np.zeros((P, 2), np.float32)
            bn[: co1 - co0, 0] = scale[co0:co1]
            bn[: co1 - co0, 1] = shift[co0:co1]
            self.bns.append(bn)

    def upw(self, s, h):
        key = (s, h)
        if key in self.uw:
            return self.uw[key]
        oh = s * h
        cols = np.zeros((3 * s, h), np.float32)
        sc = (h - 1.0) / (oh - 1.0)
        for p in range(s):
            for kk in range(h):
                o = s * kk + p
                pos = o * sc
                x0 = int(np.floor(pos))
                f = pos - x0
                cols[p * 3 + (x0 - kk + 1), kk] += 1.0 - f
                if f > 0:
                    cols[p * 3 + (x0 + 1 - kk + 1), kk] += f
        off = self.uwlen
        self.uwsegs.append(cols.astype(np.float16))
        self.uwlen += cols.size
        self.uw[key] = off
        return off


def build(params, col, nc=None):
    import concourse.mybir as mybir
    emit = nc is not None
    if emit:
        pools = nc._pools
        F16 = mybir.dt.float16
        F32 = mybir.dt.float32
        AF = mybir.ActivationFunctionType
        OP = mybir.AluOpType

    C = 32

    TAGMAP = {"xim_0": "sA", "houtb_0": "sA",
              "s1out_0": "sB", "houtb_1": "sB",
              "l1in_0": "sC", "hout_0": "sC",
              "l1ds_0": "x0a_0", "l1ds_1": "x0b_0",
              "upu_0": "bt0_0", "upu_1": "bt0_1",
              "upx_0": "upxt", "facc0_0": "facc", "facc1_0": "facc",
              "facc2_0": "facc", "facc3_0": "facc", "facc3_1": "faccb"}

    class Act:
        def __init__(self, name, Cch, H, padded=True, dtype="f16"):
            self.C, self.H, self.W = Cch, H, H
            self.padded = padded
            self.nct = (Cch + P - 1) // P
            self.tiles = None
            if emit:
                dt = F16 if dtype == "f16" else F32
                hp = H + 2 if padded else H
                self.tiles = []
                for ct in range(self.nct):
                    cc = min(P, Cch - ct * P)
                    tg = TAGMAP.get(f"{name}_{ct}", f"{name}_{ct}")
                    self.tiles.append(
                        pools["acts"].tile([cc, hp, hp], dt, tag=tg,
                                           name=f"{name}_{ct}"))

        def zero_pads(self):
            if not emit or not self.padded:
                return
            hp = self.H + 2
            for t in self.tiles:
                nc.vector.memset(t[:, 0:1, :], 0.0)
                nc.vector.memset(t[:, hp - 1:hp, :], 0.0)
                nc.vector.memset(t[:, :, 0:1], 0.0)
                nc.vector.memset(t[:, :, hp - 1:hp], 0.0)

        def rv(self, ct, r0, R, ky, kx, s, Wo):
            t = self.tiles[ct]
            if not self.padded:
                return t[:, r0:r0 + R, 0:Wo]
            if s > 1:
                return t[:, ky + s * r0: ky + s * (r0 + R): s,
                         kx: kx + s * Wo: s]
            return t[:, ky + r0: ky + r0 + R, kx: kx + Wo]

        def interior(self, ct, r0=0, R=None):
            R = R if R is not None else self.H
            t = self.tiles[ct]
            if self.padded:
                return t[:, 1 + r0:1 + r0 + R, 1:1 + self.W]
            return t[:, r0:r0 + R, :]

    def act(name, Cch, H, padded=True, dtype="f16"):
        a = Act(name, Cch, H, padded, dtype)
        a.zero_pads()
        return a

    def conv(src, dst, w, bnp, k, stride, relu=True, res=None, accum=None):
        w = _np(w)
        Cout, Cin = w.shape[0], w.shape[1]
        widx, bslots = col.conv(w, Cout, Cin, k)
        if not col.pass2:
            col.bn(bnp['g'], bnp['b'], bnp['m'], bnp['v'], Cout)
        if not emit:
            return
        tgt = dst if dst is not None else accum
        Wo, Ho = tgt.W, tgt.H
        Rch = min(Ho, max(1, 512 // Wo))
        base = 1 if k == 1 else 0  # 1x1 reads interior of padded bufs
        for mt in range((Cout + P - 1) // P):
            cc = min(P, Cout - mt * P)
            wts = {}
            for (m2, ky, kx, ct), wi in sorted(widx.items()):
                if m2 != mt:
                    continue
                wt = pools["w"].tile([P, P], F16, tag="w", name="wt")
                nc.sync.dma_start(out=wt[:], in_=nc._wblob[wi, :, :])
                wts[(ky, kx, ct)] = wt
            bsl = bslots[mt]
            scale = nc._bn_sb[:cc, bsl, 0:1]
            shift = nc._bn_sb[:cc, bsl, 1:2]
            for r0 in range(0, Ho, Rch):
                R = min(Rch, Ho - r0)
                ps = pools["ps"].tile([cc, R, Wo], F32, tag="ps", name="ps")
                keys = sorted(wts.keys())
                for i, (ky, kx, ct) in enumerate(keys):
                    cinc = min(P, Cin - ct * P)
                    rhs = src.rv(ct, r0, R, ky + base, kx + base, stride, Wo) \
                        if src.padded else src.rv(ct, r0, R, ky, kx, stride, Wo)
                    nc.tensor.matmul(ps[:], wts[(ky, kx, ct)][:cinc, :cc], rhs,
                                     start=(i == 0), stop=(i == len(keys) - 1))
                if accum is not None:
                    tmp = pools["t"].tile([cc, R, Wo], F32, tag="tf", name="tmpf")
                    nc.vector.tensor_scalar(tmp[:], ps[:], scale, shift,
                                            OP.mult, OP.add)
                    av = accum.tiles[mt][:, r0:r0 + R, :]
                    nc.vector.tensor_tensor(av, av, tmp[:], OP.add)
                elif res is not None:
                    tmp = pools["t"].tile([cc, R, Wo], F16, tag="tt", name="tmph")
                    nc.vector.tensor_scalar(tmp[:], ps[:], scale, shift,
                                            OP.mult, OP.add)
                    nc.vector.tensor_tensor(tmp[:], tmp[:],
                                            res.interior(mt, r0, R), OP.add)
                    nc.scalar.activation(dst.interior(mt, r0, R), tmp[:], AF.Relu)
                else:
                    nc.scalar.activation(dst.interior(mt, r0, R), ps[:],
                                         AF.Relu if relu else AF.Identity,
                                         bias=shift, scale=scale)

    def upsample(src, s, dst_fp32=None):
        h = src.H
        oh = s * h
        off = col.upw(s, h)
        if not emit:
            return None
        uw = nc._uw_sb
        out = Act("upu", src.C, oh, padded=False) if dst_fp32 is None else None
        for ct in range(src.nct):
            cc = min(P, src.C - ct * P)
            upx = pools["acts"].tile([cc, h + 2, oh], F16, tag="upxt", name="upx")
            nc.vector.memset(upx[:, 0:1, :], 0.0)
            nc.vector.memset(upx[:, h + 1:h + 2, :], 0.0)
            st = src.tiles[ct]
            for p in range(s):
                ov = upx[:, 1:1 + h, p::s]
                sc = pools["t"].tile([cc, h, h], F16, tag="upsc", name="upsc")
                for di in range(3):
                    o2 = off + (p * 3 + di) * h
                    wb = uw[:cc, o2:o2 + h][:, None, :].to_broadcast((cc, h, h))
                    iv = st[:, 1:1 + h, di:di + h]
                    if di == 0:
                        nc.vector.tensor_tensor(ov, iv, wb, OP.mult)
                    else:
                        nc.vector.tensor_tensor(sc[:], iv, wb, OP.mult)
                        nc.vector.tensor_tensor(ov, ov, sc[:], OP.add)
            ot = out.tiles[ct] if out is not None else dst_fp32[ct]
            for p in range(s):
                ov = ot[:, p::s, :]
                sc = pools["t"].tile([cc, h, oh], F16, tag="upscy", name="upscy")
                for di in range(3):
                    o2 = off + (p * 3 + di) * h
                    wb = uw[:cc, o2:o2 + h][:, :, None].to_broadcast((cc, h, oh))
                    iv = upx[:, di:di + h, :]
                    if di == 0:
                        nc.vector.tensor_tensor(ov, iv, wb, OP.mult)
                    else:
                        nc.vector.tensor_tensor(sc[:], iv, wb, OP.mult)
                        nc.vector.tensor_tensor(ov, ov, sc[:], OP.add)
        return out

    # ---------------- network ----------------
    st = params['stem']
    x_im = act("xim", 27, 112)
    if emit:
        nc.sync.dma_start(out=x_im.tiles[0][:, :, :], in_=nc._xim[:, :, :])
    s1 = act("s1out", 64, 112)
    w1 = np.transpose(_np(st['c1']), (0, 2, 3, 1)).reshape(64, 27, 1, 1)
    conv(x_im, s1, w1, st['bn1'], 1, 1)
    l1in = act("l1in", 64, 56)
    conv(s1, l1in, st['c2'], st['bn2'], 3, 2)

    l1a = act("l1a", 256, 56)
    l1b = act("l1b", 256, 56)
    bta = act("bta", 64, 56)
    btb = act("btb", 64, 56)
    l1ds = act("l1ds", 256, 56)
    cur_ = l1in
    for bi, bp in enumerate(params['layer1']):
        dst = [l1a, l1b][bi % 2]
        conv(cur_, bta, bp['c1'], bp['bn1'], 1, 1)
        conv(bta, btb, bp['c2'], bp['bn2'], 3, 1)
        if 'ds_c' in bp:
            conv(cur_, l1ds, bp['ds_c'], bp['ds_bn'], 1, 1, relu=False)
            res = l1ds
        else:
            res = cur_
        conv(btb, dst, bp['c3'], bp['bn3'], 1, 1, res=res)
        cur_ = dst
    l1out = cur_

    chs = [C, 2 * C, 4 * C, 8 * C]
    HS = [56, 28, 14, 7]
    pairs = [[act(f"x{i}a", chs[i], HS[i]), act(f"x{i}b", chs[i], HS[i])]
             for i in range(4)]
    btmp = [[act(f"bt{i}_0", chs[i], HS[i]), act(f"bt{i}_1", chs[i], HS[i])]
            for i in range(4)]
    t1 = params['trans1']
    conv(l1out, pairs[0][0], t1[0]['c'], t1[0]['bn'], 3, 1)
    conv(l1out, pairs[1][0], t1[1]['c'], t1[1]['bn'], 3, 2)
    cur = [pairs[0][0], pairs[1][0]]
    alt = [pairs[0][1], pairs[1][1]]

    def stage(sp, n):
        hs = []
        for i in range(n):
            h = cur[i]
            tA = btmp[i][0]
            for bj, bp in enumerate(sp['branches'][i]):
                conv(h, tA, bp['c1'], bp['bn1'], 3, 1)
                dst = btmp[i][1] if bj % 2 == 0 else alt[i]
                conv(tA, dst, bp['c2'], bp['bn2'], 3, 1, res=h)
                h = dst
            hs.append(h)  # == alt[i] after 4 blocks
        fp = sp['fuse']
        for j in range(n):
            acc = Act(f"facc{j}", chs[j], HS[j], padded=False, dtype="f32")
            if emit:
                for mt in range(acc.nct):
                    nc.vector.tensor_copy(acc.tiles[mt][:, :, :],
                                          hs[j].interior(mt))
            for i in range(n):
                if i == j:
                    continue
                e = fp[j][i]
                if i < j:
                    conv(hs[i], None, e['c'], e['bn'], 3, 2 ** (j - i), accum=acc)
                else:
                    up = upsample(hs[i], 2 ** (i - j))
                    conv(up if emit else hs[i], None, e['c'], e['bn'], 1, 1,
                         accum=acc)
            if emit:
                for mt in range(cur[j].nct):
                    nc.scalar.activation(cur[j].interior(mt),
                                         acc.tiles[mt][:, :, :], AF.Relu)
        # fuse outputs are in cur; alt holds branch-block results (stale)

    stage(params['stage2'], 2)
    t2 = params['trans2']
    conv(cur[1], pairs[2][0], t2['c'], t2['bn'], 3, 2)
    cur.append(pairs[2][0])
    alt.append(pairs[2][1])
    stage(params['stage3'], 3)
    t3 = params['trans3']
    conv(cur[2], pairs[3][0], t3['c'], t3['bn'], 3, 2)
    cur.append(pairs[3][0])
    alt.append(pairs[3][1])
    stage(params['stage4'], 4)

    if emit:
        h0t = pools["acts"].tile([32, 56, 56], F32, tag="sC", name="h0t")
        nc.scalar.activation(h0t[:, :, :], cur[0].interior(0), AF.Copy)
        nc.sync.dma_start(out=nc._out[0:32], in_=h0t[:, :, :])
    for i, (c0, c1) in enumerate([(32, 96), (96, 224), (224, 480)]):
        src = cur[i + 1] if emit else None
        s = 2 ** (i + 1)
        if emit:
            dsts = []
            for ct in range(src.nct):
                cc = min(P, src.C - ct * P)
                dsts.append(pools["acts"].tile([cc, 56, 56], F32,
                                               tag=TAGMAP[f"houtb_{ct}"],
                                               name=f"houtb{ct}"))
            upsample(src, s, dst_fp32=dsts)
            for ct, dt_ in enumerate(dsts):
                cc = min(P, src.C - ct * P)
                nc.sync.dma_start(out=nc._out[c0 + ct * P: c0 + ct * P + cc],
                                  in_=dt_[:, :, :])
        else:
            col.upw(s, HS[i + 1])


_CACHE = {}


def _prep(params):
    col = Col()
    build(params, col, nc=None)
    wblob = np.stack(col.wtiles)
    bnblob = np.stack(col.bns)
    segs = [a[r] for a in col.uwsegs for r in range(a.shape[0])]
    uwflat = np.concatenate(segs)
    uwrep = np.tile(uwflat[None, :], (P, 1)).astype(np.float16)
    return wblob, bnblob, uwrep


def _im2col(img):
    xp = np.pad(np.asarray(img, np.float32), ((0, 0), (1, 1), (1, 1)))
    out = np.zeros((27, 114, 114), np.float16)
    t = 0
    for ky in range(3):
        for kx in range(3):
            for c in range(3):
                out[t, 1:113, 1:113] = xp[c, ky:ky + 224:2, kx:kx + 224:2]
                t += 1
    return out


def kernel(x, params):
    import concourse.mybir as mybir
    from concourse import bacc
    from concourse.tile import TileContext
    from concourse.bass_utils import run_bass_kernel_spmd
    import contextlib

    x = np.asarray(x, dtype=np.float32)
    B = x.shape[0]
    wblob, bnblob, uwrep = _prep(params)

    if "net" not in _CACHE:
        nc = bacc.Bacc()
        nc._xim = nc.dram_tensor("xim", [27, 114, 114], mybir.dt.float16,
                                 kind="ExternalInput")
        nc._wblob = nc.dram_tensor("wblob", list(wblob.shape), mybir.dt.float16,
                                   kind="ExternalInput")
        nc._bnblob = nc.dram_tensor("bnblob", list(bnblob.shape),
                                    mybir.dt.float32, kind="ExternalInput")
        nc._uwb = nc.dram_tensor("uwb", [P, uwrep.shape[1]], mybir.dt.float16,
                                 kind="ExternalInput")
        nc._out = nc.dram_tensor("out", [480, 56, 56], mybir.dt.float32,
                                 kind="ExternalOutput")
        with TileContext(nc) as tc:
            with contextlib.ExitStack() as stack:
                pools = {}
                pools["acts"] = stack.enter_context(tc.tile_pool(name="acts", bufs=1))
                pools["w"] = stack.enter_context(tc.tile_pool(name="w", bufs=48))
                pools["ps"] = stack.enter_context(
                    tc.tile_pool(name="ps", bufs=6, space="PSUM"))
                pools["t"] = stack.enter_context(tc.tile_pool(name="t", bufs=2))
                nc._pools = pools
                NS = bnblob.shape[0]
                bn_sb = pools["acts"].tile([P, NS, 2], mybir.dt.float32, tag="bn", name="bn_sb")
                nc.sync.dma_start(out=bn_sb[:],
                                  in_=nc._bnblob[:, :, :].rearrange("n p two -> p n two"))
                nc._bn_sb = bn_sb
                uw_sb = pools["acts"].tile([P, uwrep.shape[1]], mybir.dt.float16,
                                           tag="uw", name="uw_sb")
                nc.sync.dma_start(out=uw_sb[:], in_=nc._uwb[:, :])
                nc._uw_sb = uw_sb
                col2 = Col()
                col2.pass2 = True
                build(params, col2, nc=nc)
        nc.finalize()
        _CACHE["net"] = nc
    nc = _CACHE["net"]

    in_maps = [{"xim": _im2col(x[i]), "wblob": wblob, "bnblob": bnblob,
                "uwb": uwrep} for i in range(B)]
    r = run_bass_kernel_spmd(nc, in_maps, core_ids=list(range(B)))
    feats = np.stack([r.results[i]["out"] for i in range(B)])
    return feats, feats[:, :32], feats[:, 224:]
